# revision 11
# baseline (speedup 1.0000x reference)
"""Pairwise cosine-similarity kernel for Trainium2 (8 NeuronCores, SPMD).

Computes out = 16 * normalize(x1) @ normalize(x2).T for x1, x2 [8192, 512] f32.

Sharding: 4x2 grid. x1 rows split 4 ways (2048 rows/core), x2 rows split 2
ways (4096 rows/core); each core computes a [2048, 4096] output block; the
host concatenates and upcasts bf16 -> f32.

Host-side prep is layout/dtype only:
  - x1t [512, 2048] bf16: pre-transposed x1 slice (GEMM stationary source).
  - x2t [512, 4096] bf16: pre-transposed x2 slice (GEMM moving source).
  - x1n [4, 128, 2048] fp8e4m3, x2n [8, 128, 2048] fp8e4m3: natural-layout
    row-grouped copies used only for row-norm computation
    (group g holds rows g*512 + j*128 + p at [g, p, j*512:(j+1)*512]).

All FLOPs run on device. Schedule (v2) targets zero PE dead time:
  1. DMA is chunked and deadline-ordered: x2t cg0 + the first 512 columns of
     x1t + x2n g0/g1 first, so real matmuls start ~13us (HWDGE first bytes
     land ~8.7us after NEFF start; the preamble itself runs to ~6.9us).
  2. ~12 junk fp16 matmuls fill 7.6-13us purely to lift the PE HAM clock
     gate (1.2 -> 2.4 GHz) before real work; they overlap the input DMA.
  3. cg0 (output cols 0:1024) is computed on UNSCALED x2t; its PSUM tiles
     are evicted with two ops (x bcs0 col-scale, x inv1 row-scale) spread
     across DVE/ACT. This removes all norm math from the GEMM-start
     critical path. cg1-3 use x2t pre-scaled in place on GPSIMD (idle
     engine) and single-op evictions.
  4. PSUM tiles are [128, 1024] (2 banks) so one instruction evicts a whole
     m-tile. Out-DMA triggers run on the Sync queue (not ACT). Late bulk
     inputs (x2t cg2/cg3, x2n g4-g7) go via the GPSIMD SWDGE queue so the
     Sync queue can start issuing output DMAs early.
  5. inv1 folds the 16x output scale; inv2 is partition-broadcast via fp16
     diag matmuls (ones.T @ diag(inv2)) interleaved into the GEMM stream.
"""

import sys

for _p in ("/root/.axon_site/_ro/trn_rl_repo", "/opt/trn_rl_repo"):
    if _p not in sys.path:
        sys.path.append(_p)

import ml_dtypes
import numpy as np

import concourse.bass as bass
import concourse.tile as tile
from concourse import bacc, mybir
from concourse.bass_utils import run_bass_kernel_spmd
from concourse.masks import make_identity

F32 = mybir.dt.float32
BF16 = mybir.dt.bfloat16
FP16 = mybir.dt.float16
FP8 = mybir.dt.float8e4
P = 128
SCALE = 16.0
EPS = 1e-8

N_CORES = 8
GRID_I = 4  # row-shards of x1
GRID_J = 2  # column-shards of x2
N1 = 8192
N2 = 8192
D = 512
CG = 1024  # output column-group width

_PROGRAM_CACHE = {}


def build_program(n1_local=N1 // GRID_I, n2=N2 // GRID_J, d=D):
    kc = d // P                 # 4 contraction chunks
    m_tiles = n1_local // P     # 16 row tiles per core
    n_cgs = n2 // CG            # 4 column groups
    g1 = n1_local // 512        # 4 x1 norm groups
    g2 = n2 // 512              # 8 x2 norm groups

    nc = bacc.Bacc("TRN2", target_bir_lowering=False, debug=False,
                   num_devices=N_CORES)
    x1t = nc.dram_tensor("x1t", [d, n1_local], BF16, kind="ExternalInput")
    x1n = nc.dram_tensor("x1n", [g1, P, 2048], FP8, kind="ExternalInput")
    x2n = nc.dram_tensor("x2n", [g2, P, 2048], FP8, kind="ExternalInput")
    x2t = nc.dram_tensor("x2t", [d, n2], BF16, kind="ExternalInput")
    out = nc.dram_tensor("out", [n1_local, n2], BF16, kind="ExternalOutput")

    AF = mybir.ActivationFunctionType
    ALU = mybir.AluOpType

    with tile.TileContext(nc) as tc:
        with (
            tc.tile_pool(name="const", bufs=1) as const,
            tc.tile_pool(name="xt", bufs=1) as xt,
            tc.tile_pool(name="ldn", bufs=1) as ldn,
            tc.tile_pool(name="sq", bufs=4) as sqp,
            tc.tile_pool(name="stat", bufs=1) as stat,
            tc.tile_pool(name="dg", bufs=2) as dgp,
            tc.tile_pool(name="bc", bufs=1) as bcp,
            tc.tile_pool(name="ot", bufs=6) as otp,
            tc.tile_pool(name="ps", bufs=6, space="PSUM") as psp,
            tc.tile_pool(name="psb", bufs=2, space="PSUM") as psb,
        ):
            # ---- constants (cheap memsets so PE warmup starts early) --------
            ones_h = const.tile([P, P], FP16)
            nc.gpsimd.memset(ones_h[:], 1.0)
            warm = const.tile([P, 512], FP16)
            nc.gpsimd.memset(warm[:], 0.0)
            ident4 = const.tile([P, 4, P], FP16)
            nc.gpsimd.memset(ident4[:], 0.0)
            for b in range(4):
                make_identity(nc, ident4[:, b], nomemset=True)

            # ---- SBUF input tiles -------------------------------------------
            x1n_t = [ldn.tile([P, 4, 512], FP8, tag=f"x1n_{g}",
                              name=f"x1n_{g}") for g in range(g1)]
            x2n_t = [ldn.tile([P, 4, 512], FP8, tag=f"x2n_{g}",
                              name=f"x2n_{g}") for g in range(g2)]
            # x1T split in 512-column chunks per k so the first m-tiles can
            # start before the whole 2 MB of x1t lands.
            n_c = n1_local // 512
            x1T = [[xt.tile([P, 512], BF16, tag=f"x1T_{k}_{c}",
                            name=f"x1T_{k}_{c}") for c in range(n_c)]
                   for k in range(kc)]
            x2T = [[xt.tile([P, CG], BF16, tag=f"x2T_{k}_{cg}",
                            name=f"x2T_{k}_{cg}") for cg in range(n_cgs)]
                   for k in range(kc)]

            def dma_x2T(k, cg, eng=None):
                (eng or nc.sync).dma_start(
                    x2T[k][cg][:], x2t[k * P:(k + 1) * P,
                                       cg * CG:(cg + 1) * CG])

            def dma_x1T(k, c):
                nc.sync.dma_start(
                    x1T[k][c][:], x1t[k * P:(k + 1) * P,
                                      c * 512:(c + 1) * 512])

            def dma_x2n(g, eng=None):
                (eng or nc.sync).dma_start(
                    x2n_t[g][:], x2n.ap()[g].rearrange("p (j e) -> p j e", j=4)
                )

            def dma_x1n(g):
                nc.sync.dma_start(
                    x1n_t[g][:], x1n.ap()[g].rearrange("p (j e) -> p j e", j=4)
                )

            # ---- first-wave input DMAs (sync queue, deadline order) ---------
            dma_x2T(0, 0)
            dma_x1T(0, 0)
            dma_x2n(0)
            dma_x2n(1)
            dma_x2T(1, 0)
            dma_x1T(1, 0)
            dma_x2T(2, 0)
            dma_x1T(2, 0)
            dma_x2T(3, 0)
            dma_x1T(3, 0)
            dma_x1n(0)
            for g in (1, 2, 3):
                dma_x1n(g)
            for k in range(kc):
                dma_x1T(k, 1)
            dma_x2n(2)
            dma_x2n(3)
            for k in range(kc):
                dma_x1T(k, 2)
            for k in range(kc):
                dma_x2T(k, 1)
            for k in range(kc):
                dma_x1T(k, 3)

            # ---- late bulk inputs ------------------------------------------
            for g in (4, 5, 6, 7):
                dma_x2n(g)
            for cg in (2, 3):
                for k in range(kc):
                    dma_x2T(k, cg)

            # ---- PE warmup against the HAM clock gate -----------------------
            for w in range(18):
                ps_w = psb.tile([P, 512], F32, tag="psb", name=f"warm_{w}")
                nc.tensor.matmul(ps_w[:], lhsT=ones_h[:], rhs=warm[:],
                                 start=True, stop=True)

            # ---- stats / broadcast helpers ----------------------------------
            ssq2 = [stat.tile([P, 8], F32, tag=f"ssq2_{cg}", name=f"ssq2_{cg}")
                    for cg in range(n_cgs)]
            inv2 = [stat.tile([P, 8], F32, tag=f"inv2_{cg}", name=f"inv2_{cg}")
                    for cg in range(n_cgs)]
            ssq1 = stat.tile([P, 4 * g1], F32, tag="ssq1")
            inv1 = stat.tile([P, 4 * g1], F32, tag="inv1")
            dg4s = {}
            psbs = {}
            bcs = [bcp.tile([P, CG], BF16, tag=f"bc_{cg}", name=f"bc_{cg}")
                   for cg in range(n_cgs)]

            def sq_j(src, acc, j, eng=None):
                """acc[:, j] = row sum of src[:, j]^2 (ACT Square+accum)."""
                sq_t = sqp.tile([P, 512], BF16, tag="sqa")
                nc.scalar.activation(
                    sq_t[:], src[:, j], AF.Square,
                    accum_out=acc[:, j:j + 1],
                )

            def stats_x2_group(cg, h):
                """ssq2[cg][:, 4h:4h+4] from x2n group 2*cg+h (split engines)."""
                g = 2 * cg + h
                s = ssq2[cg]
                for j in range(4):
                    sq_j(x2n_t[g], s[:, 4 * h:4 * h + 4], j,
                         "act" if j < 2 else "dve")

            def inv2_finish(cg):
                """inv2[cg] = 1 / max(row_norm, EPS) for the whole cg."""
                iv = inv2[cg][:]
                nc.scalar.activation(iv, ssq2[cg][:], AF.Sqrt)
                nc.vector.tensor_scalar_max(iv, iv, EPS)
                nc.vector.reciprocal(iv, iv)

            def stats_x1_group(g):
                """inv1[:, 4g:4g+4] = 16 / max(row_norm, EPS) (scale folded)."""
                for j in range(4):
                    sq_j(x1n_t[g], ssq1[:, 4 * g:4 * g + 4], j,
                         "act" if j < 2 else "dve")
                sl = slice(4 * g, 4 * g + 4)
                nc.scalar.activation(inv1[:, sl], ssq1[:, sl], AF.Sqrt,
                                     scale=1.0 / 256.0)
                nc.vector.tensor_scalar_max(inv1[:, sl], inv1[:, sl],
                                            EPS / 16.0)
                nc.vector.reciprocal(inv1[:, sl], inv1[:, sl])

            def dg4_build(cg, h):
                dg4 = dgp.tile([P, 4, P], FP16, tag="dg", name=f"dg_{cg}_{h}")
                nc.vector.tensor_mul(
                    dg4[:], ident4[:],
                    inv2[cg][:, 4 * h:4 * h + 4, None].to_broadcast((P, 4, P)),
                )
                dg4s[(cg, h)] = dg4

            def bcast_mm(cg, h):
                ps_b = psb.tile([P, 512], F32, tag="psb", name=f"psb_{cg}_{h}")
                nc.tensor.matmul(ps_b[:], lhsT=ones_h[:], rhs=dg4s[(cg, h)][:],
                                 start=True, stop=True)
                psbs[(cg, h)] = ps_b

            def bc_copy(cg, h):
                c0 = 4 * h * P
                nc.vector.tensor_copy(bcs[cg][:, c0:c0 + 512],
                                      psbs[(cg, h)][:])

            def scale_x2(cg, k):
                """x2T[k][cg] *= bcs[cg] in place (DVE, baseline-proven)."""
                nc.vector.tensor_mul(x2T[k][cg][:], x2T[k][cg][:],
                                     bcs[cg][:])

            def gemm_m(cg, m):
                pss = [psp.tile([P, 512], F32, tag="ps",
                                name=f"ps_{cg}_{m}_{j}") for j in range(2)]
                for k in range(kc):
                    for j in range(2):
                        nc.tensor.matmul(
                            pss[j][:],
                            lhsT=x1T[k][m // 4][:, (m % 4) * P:(m % 4 + 1) * P],
                            rhs=x2T[k][cg][:, j * 512:(j + 1) * 512],
                            start=(k == 0), stop=(k == kc - 1),
                        )
                ot = otp.tile([P, CG], BF16, tag="ot", name=f"ot_{cg}_{m}")
                iv = inv1[:, m:m + 1]
                last = (cg == n_cgs - 1) and (m == m_tiles - 1)
                # PSUM reads stay within one bank ([P, 512] halves).
                if last:
                    # split the final eviction across both engines + 2 DMAs
                    nc.vector.tensor_scalar_mul(ot[:, 0:512],
                                                pss[0][:], iv)
                    nc.scalar.activation(ot[:, 512:1024], pss[1][:],
                                         AF.Copy, scale=iv)
                    base = cg * CG
                    nc.scalar.dma_start(
                        out.ap()[m * P:(m + 1) * P, base:base + 512],
                        ot[:, 0:512])
                    nc.scalar.dma_start(
                        out.ap()[m * P:(m + 1) * P, base + 512:base + 1024],
                        ot[:, 512:1024])
                else:
                    if m % 2 == 0:
                        for j in range(2):
                            nc.vector.tensor_scalar_mul(
                                ot[:, j * 512:(j + 1) * 512], pss[j][:], iv)
                    else:
                        for j in range(2):
                            nc.scalar.activation(ot[:, j * 512:(j + 1) * 512],
                                                 pss[j][:],
                                                 AF.Copy, scale=iv)
                    nc.scalar.dma_start(
                        out.ap()[m * P:(m + 1) * P, cg * CG:(cg + 1) * CG],
                        ot[:])

            # ---- bootstrap stats for cg0 (overlaps warmup + first DMAs) -----
            # Program order defines RAW deps in the online Tile tracker, so
            # everything the cg0 evictions read (bcs[0], inv1 g0) must be
            # emitted before gemm_m(0, 0). The PE's gemm matmuls don't depend
            # on any of it, so the scheduler still overlaps them.
            for h in (0, 1):
                stats_x2_group(0, h)
            inv2_finish(0)
            for h in (0, 1):
                dg4_build(0, h)
            for h in (0, 1):
                bcast_mm(0, h)
                bc_copy(0, h)
            for k in range(kc):
                scale_x2(0, k)
            stats_x1_group(0)

            # ---- main loop --------------------------------------------------
            # per cg: 16 m-tiles; bcast matmuls for cg0 slot in after m0 (their
            # diag inputs are ready by then); stats/broadcast/scale for cg+1
            # are spread through the window.
            for cg in range(n_cgs):
                nxt = cg + 1
                for m in range(m_tiles):
                    gemm_m(cg, m)
                    if cg == 0:
                        if m == 1:
                            stats_x1_group(1)
                        elif m == 2:
                            stats_x1_group(2)
                        elif m == 3:
                            stats_x1_group(3)
                    if nxt < n_cgs:
                        if m == 5:
                            stats_x2_group(nxt, 0)
                        elif m == 6:
                            stats_x2_group(nxt, 1)
                            inv2_finish(nxt)
                        elif m == 7:
                            dg4_build(nxt, 0)
                            dg4_build(nxt, 1)
                        elif m == 9:
                            bcast_mm(nxt, 0)
                            bc_copy(nxt, 0)
                        elif m == 11:
                            bcast_mm(nxt, 1)
                            bc_copy(nxt, 1)
                            for k in range(kc):
                                scale_x2(nxt, k)

    nc.compile()
    return nc


def _get_program():
    key = "default"
    if key not in _PROGRAM_CACHE:
        _PROGRAM_CACHE[key] = build_program()
    return _PROGRAM_CACHE[key]


def _norm_groups(x8: np.ndarray) -> np.ndarray:
    """[G*512, 512] f32 -> [G, 128, 2048] fp8 with rows g*512+j*128+p."""
    g = x8.shape[0] // 512
    r = x8.reshape(g, 4, P, 512).transpose(0, 2, 1, 3).reshape(g, P, 2048)
    return np.ascontiguousarray(r.astype(ml_dtypes.float8_e4m3))


def make_in_maps(x1: np.ndarray, x2: np.ndarray) -> list:
    x1 = np.asarray(x1, dtype=np.float32)
    x2 = np.asarray(x2, dtype=np.float32)
    assert x1.shape == (N1, D) and x2.shape == (N2, D), (x1.shape, x2.shape)
    x1_b = x1.astype(ml_dtypes.bfloat16)
    x2_b = x2.astype(ml_dtypes.bfloat16)
    rows = N1 // GRID_I
    cols = N2 // GRID_J
    x1t_i = [np.ascontiguousarray(x1_b[i * rows:(i + 1) * rows].T)
             for i in range(GRID_I)]
    x1n_i = [_norm_groups(x1[i * rows:(i + 1) * rows]) for i in range(GRID_I)]
    x2t_j = [np.ascontiguousarray(x2_b[j * cols:(j + 1) * cols].T)
             for j in range(GRID_J)]
    x2n_j = [_norm_groups(x2[j * cols:(j + 1) * cols]) for j in range(GRID_J)]
    maps = []
    for c in range(N_CORES):
        i, j = c // GRID_J, c % GRID_J
        maps.append({
            "x1t": x1t_i[i],
            "x1n": x1n_i[i],
            "x2n": x2n_j[j],
            "x2t": x2t_j[j],
        })
    return maps


def kernel(x1: np.ndarray, x2: np.ndarray) -> np.ndarray:
    nc = _get_program()
    in_maps = make_in_maps(x1, x2)
    res = run_bass_kernel_spmd(nc, in_maps, core_ids=list(range(N_CORES)))
    rows = N1 // GRID_I
    cols = N2 // GRID_J
    full = np.empty((N1, N2), dtype=np.float32)
    for c in range(N_CORES):
        i, j = c // GRID_J, c % GRID_J
        full[i * rows:(i + 1) * rows, j * cols:(j + 1) * cols] = \
            res.results[c]["out"]
    return full


if __name__ == "__main__":
    rng = np.random.default_rng(0)
    a = rng.standard_normal((N1, D), dtype=np.float32)
    b = rng.standard_normal((N2, D), dtype=np.float32)
    got = kernel(a, b)
    n1 = np.maximum(np.linalg.norm(a, axis=-1, keepdims=True), EPS)
    n2 = np.maximum(np.linalg.norm(b, axis=-1, keepdims=True), EPS)
    want = SCALE * (a / n1) @ (b / n2).T
    err = np.abs(got - want)
    rel = np.linalg.norm(got - want) / np.linalg.norm(want)
    print(f"max abs err: {err.max():.3e}  rel: {rel:.3e}")


# revision 12
# speedup vs baseline: 1.0302x; 1.0302x over previous
"""Pairwise cosine-similarity kernel for Trainium2 (8 NeuronCores, SPMD).

Computes out = 16 * normalize(x1) @ normalize(x2).T for x1, x2 [8192, 512] f32.

Sharding: 4x2 grid. x1 rows split 4 ways (2048 rows/core), x2 rows split 2
ways (4096 rows/core); each core computes a [2048, 4096] output block; the
host concatenates and upcasts bf16 -> f32.

Host-side prep is layout/dtype only:
  - x1t [512, 2048] bf16: pre-transposed x1 slice (GEMM stationary source).
  - x2t [512, 4096] bf16: pre-transposed x2 slice (GEMM moving source).
  - x1n [4, 128, 2048] fp8e4m3, x2n [8, 128, 2048] fp8e4m3: natural-layout
    row-grouped copies used only for row-norm computation
    (group g holds rows g*512 + j*128 + p at [g, p, j*512:(j+1)*512]).

All FLOPs run on device. Schedule (v2) targets zero PE dead time:
  1. DMA is chunked and deadline-ordered: x2t cg0 + the first 512 columns of
     x1t + x2n g0/g1 first, so real matmuls start ~13us (HWDGE first bytes
     land ~8.7us after NEFF start; the preamble itself runs to ~6.9us).
  2. ~12 junk fp16 matmuls fill 7.6-13us purely to lift the PE HAM clock
     gate (1.2 -> 2.4 GHz) before real work; they overlap the input DMA.
  3. cg0 (output cols 0:1024) is computed on UNSCALED x2t; its PSUM tiles
     are evicted with two ops (x bcs0 col-scale, x inv1 row-scale) spread
     across DVE/ACT. This removes all norm math from the GEMM-start
     critical path. cg1-3 use x2t pre-scaled in place on GPSIMD (idle
     engine) and single-op evictions.
  4. PSUM tiles are [128, 1024] (2 banks) so one instruction evicts a whole
     m-tile. Out-DMA triggers run on the Sync queue (not ACT). Late bulk
     inputs (x2t cg2/cg3, x2n g4-g7) go via the GPSIMD SWDGE queue so the
     Sync queue can start issuing output DMAs early.
  5. inv1 folds the 16x output scale; inv2 is partition-broadcast via fp16
     diag matmuls (ones.T @ diag(inv2)) interleaved into the GEMM stream.
"""

import sys

for _p in ("/root/.axon_site/_ro/trn_rl_repo", "/opt/trn_rl_repo"):
    if _p not in sys.path:
        sys.path.append(_p)

import ml_dtypes
import numpy as np

import concourse.bass as bass
import concourse.tile as tile
from concourse import bacc, mybir
from concourse.bass_utils import run_bass_kernel_spmd
from concourse.masks import make_identity

F32 = mybir.dt.float32
BF16 = mybir.dt.bfloat16
FP16 = mybir.dt.float16
FP8 = mybir.dt.float8e4
P = 128
SCALE = 16.0
EPS = 1e-8

N_CORES = 8
GRID_I = 4  # row-shards of x1
GRID_J = 2  # column-shards of x2
N1 = 8192
N2 = 8192
D = 512
CG = 1024  # output column-group width

_PROGRAM_CACHE = {}


def build_program(n1_local=N1 // GRID_I, n2=N2 // GRID_J, d=D):
    kc = d // P                 # 4 contraction chunks
    m_tiles = n1_local // P     # 16 row tiles per core
    n_cgs = n2 // CG            # 4 column groups
    g1 = n1_local // 512        # 4 x1 norm groups
    g2 = n2 // 512              # 8 x2 norm groups

    nc = bacc.Bacc("TRN2", target_bir_lowering=False, debug=False,
                   num_devices=N_CORES)
    x1t = nc.dram_tensor("x1t", [d, n1_local], BF16, kind="ExternalInput")
    x1n = nc.dram_tensor("x1n", [g1, P, 2048], FP8, kind="ExternalInput")
    x2n = nc.dram_tensor("x2n", [g2, P, 2048], FP8, kind="ExternalInput")
    x2t = nc.dram_tensor("x2t", [d, n2], BF16, kind="ExternalInput")
    out = nc.dram_tensor("out", [n1_local, n2], BF16, kind="ExternalOutput")

    AF = mybir.ActivationFunctionType
    ALU = mybir.AluOpType

    with tile.TileContext(nc) as tc:
        with (
            tc.tile_pool(name="const", bufs=1) as const,
            tc.tile_pool(name="xt", bufs=1) as xt,
            tc.tile_pool(name="ldn", bufs=1) as ldn,
            tc.tile_pool(name="sq", bufs=4) as sqp,
            tc.tile_pool(name="stat", bufs=1) as stat,
            tc.tile_pool(name="dg", bufs=2) as dgp,
            tc.tile_pool(name="bc", bufs=1) as bcp,
            tc.tile_pool(name="ot", bufs=6) as otp,
            tc.tile_pool(name="ps", bufs=6, space="PSUM") as psp,
            tc.tile_pool(name="psb", bufs=2, space="PSUM") as psb,
        ):
            # ---- constants (cheap memsets so PE warmup starts early) --------
            ones_h = const.tile([P, P], FP16)
            nc.gpsimd.memset(ones_h[:], 1.0)
            warm = const.tile([P, 512], FP16)
            nc.gpsimd.memset(warm[:], 0.0)
            ident4 = const.tile([P, 4, P], FP16)
            nc.gpsimd.memset(ident4[:], 0.0)
            for b in range(4):
                make_identity(nc, ident4[:, b], nomemset=True)

            # ---- SBUF input tiles -------------------------------------------
            x1n_t = [ldn.tile([P, 4, 512], FP8, tag=f"x1n_{g}",
                              name=f"x1n_{g}") for g in range(g1)]
            x2n_t = [ldn.tile([P, 4, 512], FP8, tag=f"x2n_{g}",
                              name=f"x2n_{g}") for g in range(g2)]
            x1T = [xt.tile([P, n1_local], BF16, tag=f"x1T_{k}",
                           name=f"x1T_{k}") for k in range(kc)]
            x2T = [[xt.tile([P, CG], BF16, tag=f"x2T_{k}_{cg}",
                            name=f"x2T_{k}_{cg}") for cg in range(n_cgs)]
                   for k in range(kc)]

            def dma_x2T(k, cg, eng=None):
                (eng or nc.sync).dma_start(
                    x2T[k][cg][:], x2t[k * P:(k + 1) * P,
                                       cg * CG:(cg + 1) * CG])

            def dma_x1T(k):
                nc.sync.dma_start(x1T[k][:], x1t[k * P:(k + 1) * P, :])

            def dma_x2n(g, eng=None):
                (eng or nc.sync).dma_start(
                    x2n_t[g][:], x2n.ap()[g].rearrange("p (j e) -> p j e", j=4)
                )

            def dma_x1n(g):
                nc.sync.dma_start(
                    x1n_t[g][:], x1n.ap()[g].rearrange("p (j e) -> p j e", j=4)
                )

            # ---- first-wave input DMAs (sync queue, deadline order) ---------
            for k in range(kc):
                dma_x2T(k, 0)
            dma_x2n(0)
            dma_x2n(1)
            dma_x1n(0)
            for k in range(kc):
                dma_x1T(k)
            for g in (1, 2, 3):
                dma_x1n(g)
            dma_x2n(2)
            dma_x2n(3)
            for k in range(kc):
                dma_x2T(k, 1)

            # ---- late bulk inputs ------------------------------------------
            for g in (4, 5, 6, 7):
                dma_x2n(g)
            for cg in (2, 3):
                for k in range(kc):
                    dma_x2T(k, cg)

            # ---- PE warmup against the HAM clock gate -----------------------
            for w in range(18):
                ps_w = psb.tile([P, 512], F32, tag="psb", name=f"warm_{w}")
                nc.tensor.matmul(ps_w[:], lhsT=ones_h[:], rhs=warm[:],
                                 start=True, stop=True)

            # ---- stats / broadcast helpers ----------------------------------
            ssq2 = [stat.tile([P, 8], F32, tag=f"ssq2_{cg}", name=f"ssq2_{cg}")
                    for cg in range(n_cgs)]
            inv2 = [stat.tile([P, 8], F32, tag=f"inv2_{cg}", name=f"inv2_{cg}")
                    for cg in range(n_cgs)]
            ssq1 = stat.tile([P, 4 * g1], F32, tag="ssq1")
            inv1 = stat.tile([P, 4 * g1], F32, tag="inv1")
            dg4s = {}
            psbs = {}
            bcs = [bcp.tile([P, CG], BF16, tag=f"bc_{cg}", name=f"bc_{cg}")
                   for cg in range(n_cgs)]

            def sq_j(src, acc, j, eng=None):
                """acc[:, j] = row sum of src[:, j]^2 (ACT Square+accum)."""
                sq_t = sqp.tile([P, 512], BF16, tag="sqa")
                nc.scalar.activation(
                    sq_t[:], src[:, j], AF.Square,
                    accum_out=acc[:, j:j + 1],
                )

            def stats_x2_group(cg, h):
                """ssq2[cg][:, 4h:4h+4] from x2n group 2*cg+h (split engines)."""
                g = 2 * cg + h
                s = ssq2[cg]
                for j in range(4):
                    sq_j(x2n_t[g], s[:, 4 * h:4 * h + 4], j,
                         "act" if j < 2 else "dve")

            def inv2_finish(cg):
                """inv2[cg] = 1 / max(row_norm, EPS) for the whole cg."""
                iv = inv2[cg][:]
                nc.scalar.activation(iv, ssq2[cg][:], AF.Sqrt)
                nc.vector.tensor_scalar_max(iv, iv, EPS)
                nc.vector.reciprocal(iv, iv)

            def stats_x1_group(g):
                """inv1[:, 4g:4g+4] = 16 / max(row_norm, EPS) (scale folded)."""
                for j in range(4):
                    sq_j(x1n_t[g], ssq1[:, 4 * g:4 * g + 4], j,
                         "act" if j < 2 else "dve")
                sl = slice(4 * g, 4 * g + 4)
                nc.scalar.activation(inv1[:, sl], ssq1[:, sl], AF.Sqrt,
                                     scale=1.0 / 256.0)
                nc.vector.tensor_scalar_max(inv1[:, sl], inv1[:, sl],
                                            EPS / 16.0)
                nc.vector.reciprocal(inv1[:, sl], inv1[:, sl])

            def dg4_build(cg, h):
                dg4 = dgp.tile([P, 4, P], FP16, tag="dg", name=f"dg_{cg}_{h}")
                nc.vector.tensor_mul(
                    dg4[:], ident4[:],
                    inv2[cg][:, 4 * h:4 * h + 4, None].to_broadcast((P, 4, P)),
                )
                dg4s[(cg, h)] = dg4

            def bcast_mm(cg, h):
                ps_b = psb.tile([P, 512], F32, tag="psb", name=f"psb_{cg}_{h}")
                nc.tensor.matmul(ps_b[:], lhsT=ones_h[:], rhs=dg4s[(cg, h)][:],
                                 start=True, stop=True)
                psbs[(cg, h)] = ps_b

            def bc_copy(cg, h):
                c0 = 4 * h * P
                nc.vector.tensor_copy(bcs[cg][:, c0:c0 + 512],
                                      psbs[(cg, h)][:])

            def scale_x2(cg, k):
                """x2T[k][cg] *= bcs[cg] in place (DVE, baseline-proven)."""
                nc.vector.tensor_mul(x2T[k][cg][:], x2T[k][cg][:],
                                     bcs[cg][:])

            def gemm_m(cg, m):
                pss = [psp.tile([P, 512], F32, tag="ps",
                                name=f"ps_{cg}_{m}_{j}") for j in range(2)]
                for k in range(kc):
                    for j in range(2):
                        nc.tensor.matmul(
                            pss[j][:],
                            lhsT=x1T[k][:, m * P:(m + 1) * P],
                            rhs=x2T[k][cg][:, j * 512:(j + 1) * 512],
                            start=(k == 0), stop=(k == kc - 1),
                        )
                ot = otp.tile([P, CG], BF16, tag="ot", name=f"ot_{cg}_{m}")
                iv = inv1[:, m:m + 1]
                last = (cg == n_cgs - 1) and (m == m_tiles - 1)
                # PSUM reads stay within one bank ([P, 512] halves).
                if last:
                    # split the final eviction across both engines + 2 DMAs
                    nc.vector.tensor_scalar_mul(ot[:, 0:512],
                                                pss[0][:], iv)
                    nc.scalar.activation(ot[:, 512:1024], pss[1][:],
                                         AF.Copy, scale=iv)
                    base = cg * CG
                    nc.scalar.dma_start(
                        out.ap()[m * P:(m + 1) * P, base:base + 512],
                        ot[:, 0:512])
                    nc.scalar.dma_start(
                        out.ap()[m * P:(m + 1) * P, base + 512:base + 1024],
                        ot[:, 512:1024])
                else:
                    if m % 2 == 0:
                        for j in range(2):
                            nc.vector.tensor_scalar_mul(
                                ot[:, j * 512:(j + 1) * 512], pss[j][:], iv)
                    else:
                        for j in range(2):
                            nc.scalar.activation(ot[:, j * 512:(j + 1) * 512],
                                                 pss[j][:],
                                                 AF.Copy, scale=iv)
                    nc.scalar.dma_start(
                        out.ap()[m * P:(m + 1) * P, cg * CG:(cg + 1) * CG],
                        ot[:])

            # ---- bootstrap stats for cg0 (overlaps warmup + first DMAs) -----
            # Program order defines RAW deps in the online Tile tracker, so
            # everything the cg0 evictions read (bcs[0], inv1 g0) must be
            # emitted before gemm_m(0, 0). The PE's gemm matmuls don't depend
            # on any of it, so the scheduler still overlaps them.
            for h in (0, 1):
                stats_x2_group(0, h)
            inv2_finish(0)
            for h in (0, 1):
                dg4_build(0, h)
            for h in (0, 1):
                bcast_mm(0, h)
                bc_copy(0, h)
            for k in range(kc):
                scale_x2(0, k)
            stats_x1_group(0)

            # ---- main loop --------------------------------------------------
            # per cg: 16 m-tiles; bcast matmuls for cg0 slot in after m0 (their
            # diag inputs are ready by then); stats/broadcast/scale for cg+1
            # are spread through the window.
            for cg in range(n_cgs):
                nxt = cg + 1
                for m in range(m_tiles):
                    gemm_m(cg, m)
                    if cg == 0:
                        if m == 1:
                            stats_x1_group(1)
                        elif m == 2:
                            stats_x1_group(2)
                        elif m == 3:
                            stats_x1_group(3)
                    if nxt < n_cgs:
                        if m == 5:
                            stats_x2_group(nxt, 0)
                        elif m == 6:
                            stats_x2_group(nxt, 1)
                            inv2_finish(nxt)
                        elif m == 7:
                            dg4_build(nxt, 0)
                            dg4_build(nxt, 1)
                        elif m == 9:
                            bcast_mm(nxt, 0)
                            bc_copy(nxt, 0)
                        elif m == 11:
                            bcast_mm(nxt, 1)
                            bc_copy(nxt, 1)
                            for k in range(kc):
                                scale_x2(nxt, k)

    nc.compile()
    return nc


def _get_program():
    key = "default"
    if key not in _PROGRAM_CACHE:
        _PROGRAM_CACHE[key] = build_program()
    return _PROGRAM_CACHE[key]


def _norm_groups(x8: np.ndarray) -> np.ndarray:
    """[G*512, 512] f32 -> [G, 128, 2048] fp8 with rows g*512+j*128+p."""
    g = x8.shape[0] // 512
    r = x8.reshape(g, 4, P, 512).transpose(0, 2, 1, 3).reshape(g, P, 2048)
    return np.ascontiguousarray(r.astype(ml_dtypes.float8_e4m3))


def make_in_maps(x1: np.ndarray, x2: np.ndarray) -> list:
    x1 = np.asarray(x1, dtype=np.float32)
    x2 = np.asarray(x2, dtype=np.float32)
    assert x1.shape == (N1, D) and x2.shape == (N2, D), (x1.shape, x2.shape)
    x1_b = x1.astype(ml_dtypes.bfloat16)
    x2_b = x2.astype(ml_dtypes.bfloat16)
    rows = N1 // GRID_I
    cols = N2 // GRID_J
    x1t_i = [np.ascontiguousarray(x1_b[i * rows:(i + 1) * rows].T)
             for i in range(GRID_I)]
    x1n_i = [_norm_groups(x1[i * rows:(i + 1) * rows]) for i in range(GRID_I)]
    x2t_j = [np.ascontiguousarray(x2_b[j * cols:(j + 1) * cols].T)
             for j in range(GRID_J)]
    x2n_j = [_norm_groups(x2[j * cols:(j + 1) * cols]) for j in range(GRID_J)]
    maps = []
    for c in range(N_CORES):
        i, j = c // GRID_J, c % GRID_J
        maps.append({
            "x1t": x1t_i[i],
            "x1n": x1n_i[i],
            "x2n": x2n_j[j],
            "x2t": x2t_j[j],
        })
    return maps


def kernel(x1: np.ndarray, x2: np.ndarray) -> np.ndarray:
    nc = _get_program()
    in_maps = make_in_maps(x1, x2)
    res = run_bass_kernel_spmd(nc, in_maps, core_ids=list(range(N_CORES)))
    rows = N1 // GRID_I
    cols = N2 // GRID_J
    full = np.empty((N1, N2), dtype=np.float32)
    for c in range(N_CORES):
        i, j = c // GRID_J, c % GRID_J
        full[i * rows:(i + 1) * rows, j * cols:(j + 1) * cols] = \
            res.results[c]["out"]
    return full


if __name__ == "__main__":
    rng = np.random.default_rng(0)
    a = rng.standard_normal((N1, D), dtype=np.float32)
    b = rng.standard_normal((N2, D), dtype=np.float32)
    got = kernel(a, b)
    n1 = np.maximum(np.linalg.norm(a, axis=-1, keepdims=True), EPS)
    n2 = np.maximum(np.linalg.norm(b, axis=-1, keepdims=True), EPS)
    want = SCALE * (a / n1) @ (b / n2).T
    err = np.abs(got - want)
    rel = np.linalg.norm(got - want) / np.linalg.norm(want)
    print(f"max abs err: {err.max():.3e}  rel: {rel:.3e}")


# revision 14
# speedup vs baseline: 1.0636x; 1.0324x over previous
"""Pairwise cosine-similarity kernel for Trainium2 (8 NeuronCores, SPMD).

Computes out = 16 * normalize(x1) @ normalize(x2).T for x1, x2 [8192, 512] f32.

Sharding: 4x2 grid. x1 rows split 4 ways (2048 rows/core), x2 rows split 2
ways (4096 rows/core); each core computes a [2048, 4096] output block; the
host concatenates and upcasts bf16 -> f32.

Host-side prep is layout/dtype only:
  - x1t [512, 2048] bf16: pre-transposed x1 slice (GEMM stationary source).
  - x2t [512, 4096] bf16: pre-transposed x2 slice (GEMM moving source).
  - x1n [4, 128, 2048] fp8e4m3, x2n [8, 128, 2048] fp8e4m3: natural-layout
    row-grouped copies used only for row-norm computation
    (group g holds rows g*512 + j*128 + p at [g, p, j*512:(j+1)*512]).

All FLOPs run on device. Schedule (v2) targets zero PE dead time:
  1. DMA is chunked and deadline-ordered: x2t cg0 + the first 512 columns of
     x1t + x2n g0/g1 first, so real matmuls start ~13us (HWDGE first bytes
     land ~8.7us after NEFF start; the preamble itself runs to ~6.9us).
  2. ~12 junk fp16 matmuls fill 7.6-13us purely to lift the PE HAM clock
     gate (1.2 -> 2.4 GHz) before real work; they overlap the input DMA.
  3. cg0 (output cols 0:1024) is computed on UNSCALED x2t; its PSUM tiles
     are evicted with two ops (x bcs0 col-scale, x inv1 row-scale) spread
     across DVE/ACT. This removes all norm math from the GEMM-start
     critical path. cg1-3 use x2t pre-scaled in place on GPSIMD (idle
     engine) and single-op evictions.
  4. PSUM tiles are [128, 1024] (2 banks) so one instruction evicts a whole
     m-tile. Out-DMA triggers run on the Sync queue (not ACT). Late bulk
     inputs (x2t cg2/cg3, x2n g4-g7) go via the GPSIMD SWDGE queue so the
     Sync queue can start issuing output DMAs early.
  5. inv1 folds the 16x output scale; inv2 is partition-broadcast via fp16
     diag matmuls (ones.T @ diag(inv2)) interleaved into the GEMM stream.
"""

import sys

for _p in ("/root/.axon_site/_ro/trn_rl_repo", "/opt/trn_rl_repo"):
    if _p not in sys.path:
        sys.path.append(_p)

import ml_dtypes
import numpy as np

import concourse.bass as bass
import concourse.tile as tile
from concourse import bacc, mybir
from concourse.bass_utils import run_bass_kernel_spmd
from concourse.masks import make_identity

F32 = mybir.dt.float32
BF16 = mybir.dt.bfloat16
FP16 = mybir.dt.float16
FP8 = mybir.dt.float8e4
P = 128
SCALE = 16.0
EPS = 1e-8

N_CORES = 8
GRID_I = 4  # row-shards of x1
GRID_J = 2  # column-shards of x2
N1 = 8192
N2 = 8192
D = 512
CG = 1024  # output column-group width

_PROGRAM_CACHE = {}


def build_program(n1_local=N1 // GRID_I, n2=N2 // GRID_J, d=D):
    kc = d // P                 # 4 contraction chunks
    m_tiles = n1_local // P     # 16 row tiles per core
    n_cgs = n2 // CG            # 4 column groups
    g1 = n1_local // 512        # 4 x1 norm groups
    g2 = n2 // 512              # 8 x2 norm groups

    nc = bacc.Bacc("TRN2", target_bir_lowering=False, debug=False,
                   num_devices=N_CORES)
    x1t = nc.dram_tensor("x1t", [d, n1_local], BF16, kind="ExternalInput")
    x1n = nc.dram_tensor("x1n", [g1, P, 2048], FP8, kind="ExternalInput")
    x2n = nc.dram_tensor("x2n", [g2, P, 2048], FP8, kind="ExternalInput")
    x2t = nc.dram_tensor("x2t", [d, n2], BF16, kind="ExternalInput")
    out = nc.dram_tensor("out", [n1_local, n2], BF16, kind="ExternalOutput")

    AF = mybir.ActivationFunctionType
    ALU = mybir.AluOpType

    with tile.TileContext(nc) as tc:
        with (
            tc.tile_pool(name="const", bufs=1) as const,
            tc.tile_pool(name="xt", bufs=1) as xt,
            tc.tile_pool(name="ldn", bufs=1) as ldn,
            tc.tile_pool(name="sq", bufs=4) as sqp,
            tc.tile_pool(name="stat", bufs=1) as stat,
            tc.tile_pool(name="dg", bufs=2) as dgp,
            tc.tile_pool(name="bc", bufs=1) as bcp,
            tc.tile_pool(name="ot", bufs=6) as otp,
            tc.tile_pool(name="ps", bufs=6, space="PSUM") as psp,
            tc.tile_pool(name="psb", bufs=2, space="PSUM") as psb,
        ):
            # ---- constants (cheap memsets so PE warmup starts early) --------
            ones_h = const.tile([P, P], FP16)
            nc.gpsimd.memset(ones_h[:], 1.0)
            warm = const.tile([P, 512], FP16)
            nc.gpsimd.memset(warm[:], 0.0)
            ident4 = const.tile([P, 4, P], FP16)
            nc.gpsimd.memset(ident4[:], 0.0)
            for b in range(4):
                make_identity(nc, ident4[:, b], nomemset=True)

            # ---- SBUF input tiles -------------------------------------------
            x1n_t = [ldn.tile([P, 4, 512], FP8, tag=f"x1n_{g}",
                              name=f"x1n_{g}") for g in range(g1)]
            x2n_t = [ldn.tile([P, 4, 512], FP8, tag=f"x2n_{g}",
                              name=f"x2n_{g}") for g in range(g2)]
            x1T = [xt.tile([P, n1_local], BF16, tag=f"x1T_{k}",
                           name=f"x1T_{k}") for k in range(kc)]
            x2T = [[xt.tile([P, CG], BF16, tag=f"x2T_{k}_{cg}",
                            name=f"x2T_{k}_{cg}") for cg in range(n_cgs)]
                   for k in range(kc)]

            def dma_x2T(k, cg, eng=None):
                (eng or nc.sync).dma_start(
                    x2T[k][cg][:], x2t[k * P:(k + 1) * P,
                                       cg * CG:(cg + 1) * CG])

            def dma_x1T(k):
                nc.sync.dma_start(x1T[k][:], x1t[k * P:(k + 1) * P, :])

            def dma_x2n(g, eng=None):
                (eng or nc.sync).dma_start(
                    x2n_t[g][:], x2n.ap()[g].rearrange("p (j e) -> p j e", j=4)
                )

            def dma_x1n(g):
                nc.sync.dma_start(
                    x1n_t[g][:], x1n.ap()[g].rearrange("p (j e) -> p j e", j=4)
                )

            # ---- first-wave input DMAs (sync queue, deadline order) ---------
            dma_x2n(0)
            dma_x2n(1)
            for k in range(kc):
                dma_x2T(k, 0)
            for k in range(kc):
                dma_x1T(k)
            dma_x1n(0)
            dma_x1n(1)
            dma_x2n(2)
            dma_x2n(3)
            dma_x1n(2)
            dma_x1n(3)
            for k in range(kc):
                dma_x2T(k, 1)

            # ---- late bulk inputs ------------------------------------------
            for g in (4, 5, 6, 7):
                dma_x2n(g)
            for cg in (2, 3):
                for k in range(kc):
                    dma_x2T(k, cg)

            # ---- PE warmup against the HAM clock gate -----------------------
            for w in range(30):
                ps_w = psb.tile([P, 512], F32, tag="psb", name=f"warm_{w}")
                nc.tensor.matmul(ps_w[:], lhsT=ones_h[:], rhs=warm[:],
                                 start=True, stop=True)

            # ---- stats / broadcast helpers ----------------------------------
            ssq2 = [stat.tile([P, 8], F32, tag=f"ssq2_{cg}", name=f"ssq2_{cg}")
                    for cg in range(n_cgs)]
            inv2 = [stat.tile([P, 8], F32, tag=f"inv2_{cg}", name=f"inv2_{cg}")
                    for cg in range(n_cgs)]
            ssq1 = stat.tile([P, 4 * g1], F32, tag="ssq1")
            inv1 = stat.tile([P, 4 * g1], F32, tag="inv1")
            dg4s = {}
            psbs = {}
            bcs = [bcp.tile([P, CG], BF16, tag=f"bc_{cg}", name=f"bc_{cg}")
                   for cg in range(n_cgs)]

            def sq_j(src, acc, j, eng="act"):
                """acc[:, j] = row sum of src[:, j]^2 on the given engine."""
                if eng == "act":
                    sq_t = sqp.tile([P, 512], BF16, tag="sqa")
                    nc.scalar.activation(
                        sq_t[:], src[:, j], AF.Square,
                        accum_out=acc[:, j:j + 1],
                    )
                else:
                    sq_t = sqp.tile([P, 512], BF16, tag="sqv")
                    nc.vector.tensor_mul(sq_t[:], src[:, j], src[:, j])
                    nc.vector.tensor_reduce(
                        acc[:, j:j + 1], sq_t[:], op=ALU.add,
                        axis=mybir.AxisListType.X,
                    )

            def stats_x2_group(cg, h):
                """ssq2[cg][:, 4h:4h+4] from x2n group 2*cg+h (split engines)."""
                g = 2 * cg + h
                s = ssq2[cg]
                for j in range(4):
                    sq_j(x2n_t[g], s[:, 4 * h:4 * h + 4], j,
                         "act" if j < 2 else "dve")

            def inv2_finish(cg):
                """inv2[cg] = 1 / max(row_norm, EPS) for the whole cg."""
                iv = inv2[cg][:]
                nc.scalar.activation(iv, ssq2[cg][:], AF.Sqrt)
                nc.vector.tensor_scalar_max(iv, iv, EPS)
                nc.vector.reciprocal(iv, iv)

            def stats_x1_group(g):
                """inv1[:, 4g:4g+4] = 16 / max(row_norm, EPS) (scale folded)."""
                for j in range(4):
                    sq_j(x1n_t[g], ssq1[:, 4 * g:4 * g + 4], j,
                         "act" if j < 2 else "dve")
                sl = slice(4 * g, 4 * g + 4)
                nc.scalar.activation(inv1[:, sl], ssq1[:, sl], AF.Sqrt,
                                     scale=1.0 / 256.0)
                nc.vector.tensor_scalar_max(inv1[:, sl], inv1[:, sl],
                                            EPS / 16.0)
                nc.vector.reciprocal(inv1[:, sl], inv1[:, sl])

            def dg4_build(cg, h):
                dg4 = dgp.tile([P, 4, P], FP16, tag="dg", name=f"dg_{cg}_{h}")
                nc.vector.tensor_mul(
                    dg4[:], ident4[:],
                    inv2[cg][:, 4 * h:4 * h + 4, None].to_broadcast((P, 4, P)),
                )
                dg4s[(cg, h)] = dg4

            def bcast_mm(cg, h):
                ps_b = psb.tile([P, 512], F32, tag="psb", name=f"psb_{cg}_{h}")
                nc.tensor.matmul(ps_b[:], lhsT=ones_h[:], rhs=dg4s[(cg, h)][:],
                                 start=True, stop=True)
                psbs[(cg, h)] = ps_b

            def bc_copy(cg, h):
                c0 = 4 * h * P
                nc.vector.tensor_copy(bcs[cg][:, c0:c0 + 512],
                                      psbs[(cg, h)][:])

            def scale_x2(cg, k):
                """x2T[k][cg] *= bcs[cg] in place (DVE, baseline-proven)."""
                nc.vector.tensor_mul(x2T[k][cg][:], x2T[k][cg][:],
                                     bcs[cg][:])

            def gemm_m(cg, m):
                pss = [psp.tile([P, 512], F32, tag="ps",
                                name=f"ps_{cg}_{m}_{j}") for j in range(2)]
                for k in range(kc):
                    for j in range(2):
                        nc.tensor.matmul(
                            pss[j][:],
                            lhsT=x1T[k][:, m * P:(m + 1) * P],
                            rhs=x2T[k][cg][:, j * 512:(j + 1) * 512],
                            start=(k == 0), stop=(k == kc - 1),
                        )
                ot = otp.tile([P, CG], BF16, tag="ot", name=f"ot_{cg}_{m}")
                iv = inv1[:, m:m + 1]
                last = (cg == n_cgs - 1) and (m >= m_tiles - 2)
                # PSUM reads stay within one bank ([P, 512] halves).
                if last:
                    # split the final eviction across both engines + 2 DMAs
                    nc.vector.tensor_scalar_mul(ot[:, 0:512],
                                                pss[0][:], iv)
                    nc.scalar.activation(ot[:, 512:1024], pss[1][:],
                                         AF.Copy, scale=iv)
                    base = cg * CG
                    nc.sync.dma_start(
                        out.ap()[m * P:(m + 1) * P, base:base + 512],
                        ot[:, 0:512])
                    nc.sync.dma_start(
                        out.ap()[m * P:(m + 1) * P, base + 512:base + 1024],
                        ot[:, 512:1024])
                else:
                    if m % 2 == 0:
                        for j in range(2):
                            nc.vector.tensor_scalar_mul(
                                ot[:, j * 512:(j + 1) * 512], pss[j][:], iv)
                    else:
                        for j in range(2):
                            nc.scalar.activation(ot[:, j * 512:(j + 1) * 512],
                                                 pss[j][:],
                                                 AF.Copy, scale=iv)
                    nc.sync.dma_start(
                        out.ap()[m * P:(m + 1) * P, cg * CG:(cg + 1) * CG],
                        ot[:])

            # ---- bootstrap stats for cg0 (overlaps warmup + first DMAs) -----
            # Program order defines RAW deps in the online Tile tracker, so
            # everything the cg0 evictions read (bcs[0], inv1 g0) must be
            # emitted before gemm_m(0, 0). The PE's gemm matmuls don't depend
            # on any of it, so the scheduler still overlaps them.
            for h in (0, 1):
                stats_x2_group(0, h)
            inv2_finish(0)
            for h in (0, 1):
                dg4_build(0, h)
            for h in (0, 1):
                bcast_mm(0, h)
                bc_copy(0, h)
            for k in range(kc):
                scale_x2(0, k)
            stats_x1_group(0)
            if n_cgs > 1:
                for h in (0, 1):
                    stats_x2_group(1, h)
                inv2_finish(1)
                for h in (0, 1):
                    dg4_build(1, h)

            # ---- main loop --------------------------------------------------
            # per cg: 16 m-tiles; bcast matmuls for cg0 slot in after m0 (their
            # diag inputs are ready by then); stats/broadcast/scale for cg+1
            # are spread through the window.
            for cg in range(n_cgs):
                nxt = cg + 1
                for m in range(m_tiles):
                    gemm_m(cg, m)
                    if cg == 0:
                        if m == 1:
                            stats_x1_group(1)
                        elif m == 2:
                            stats_x1_group(2)
                        elif m == 3:
                            stats_x1_group(3)
                    if nxt < n_cgs:
                        if nxt + 1 < n_cgs:
                            if m == 5:
                                stats_x2_group(nxt + 1, 0)
                            elif m == 6:
                                stats_x2_group(nxt + 1, 1)
                                inv2_finish(nxt + 1)
                            elif m == 7:
                                dg4_build(nxt + 1, 0)
                                dg4_build(nxt + 1, 1)
                        if m == 9:
                            bcast_mm(nxt, 0)
                            bc_copy(nxt, 0)
                        elif m == 11:
                            bcast_mm(nxt, 1)
                            bc_copy(nxt, 1)
                            for k in range(kc):
                                scale_x2(nxt, k)

    nc.compile()
    return nc


def _get_program():
    key = "default"
    if key not in _PROGRAM_CACHE:
        _PROGRAM_CACHE[key] = build_program()
    return _PROGRAM_CACHE[key]


def _norm_groups(x8: np.ndarray) -> np.ndarray:
    """[G*512, 512] f32 -> [G, 128, 2048] fp8 with rows g*512+j*128+p."""
    g = x8.shape[0] // 512
    r = x8.reshape(g, 4, P, 512).transpose(0, 2, 1, 3).reshape(g, P, 2048)
    return np.ascontiguousarray(r.astype(ml_dtypes.float8_e4m3))


def make_in_maps(x1: np.ndarray, x2: np.ndarray) -> list:
    x1 = np.asarray(x1, dtype=np.float32)
    x2 = np.asarray(x2, dtype=np.float32)
    assert x1.shape == (N1, D) and x2.shape == (N2, D), (x1.shape, x2.shape)
    x1_b = x1.astype(ml_dtypes.bfloat16)
    x2_b = x2.astype(ml_dtypes.bfloat16)
    rows = N1 // GRID_I
    cols = N2 // GRID_J
    x1t_i = [np.ascontiguousarray(x1_b[i * rows:(i + 1) * rows].T)
             for i in range(GRID_I)]
    x1n_i = [_norm_groups(x1[i * rows:(i + 1) * rows]) for i in range(GRID_I)]
    x2t_j = [np.ascontiguousarray(x2_b[j * cols:(j + 1) * cols].T)
             for j in range(GRID_J)]
    x2n_j = [_norm_groups(x2[j * cols:(j + 1) * cols]) for j in range(GRID_J)]
    maps = []
    for c in range(N_CORES):
        i, j = c // GRID_J, c % GRID_J
        maps.append({
            "x1t": x1t_i[i],
            "x1n": x1n_i[i],
            "x2n": x2n_j[j],
            "x2t": x2t_j[j],
        })
    return maps


def kernel(x1: np.ndarray, x2: np.ndarray) -> np.ndarray:
    nc = _get_program()
    in_maps = make_in_maps(x1, x2)
    res = run_bass_kernel_spmd(nc, in_maps, core_ids=list(range(N_CORES)))
    rows = N1 // GRID_I
    cols = N2 // GRID_J
    full = np.empty((N1, N2), dtype=np.float32)
    for c in range(N_CORES):
        i, j = c // GRID_J, c % GRID_J
        full[i * rows:(i + 1) * rows, j * cols:(j + 1) * cols] = \
            res.results[c]["out"]
    return full


if __name__ == "__main__":
    rng = np.random.default_rng(0)
    a = rng.standard_normal((N1, D), dtype=np.float32)
    b = rng.standard_normal((N2, D), dtype=np.float32)
    got = kernel(a, b)
    n1 = np.maximum(np.linalg.norm(a, axis=-1, keepdims=True), EPS)
    n2 = np.maximum(np.linalg.norm(b, axis=-1, keepdims=True), EPS)
    want = SCALE * (a / n1) @ (b / n2).T
    err = np.abs(got - want)
    rel = np.linalg.norm(got - want) / np.linalg.norm(want)
    print(f"max abs err: {err.max():.3e}  rel: {rel:.3e}")
